# revision 1
# baseline (speedup 1.0000x reference)
"""Trainium2 Bass kernel for nn_BinaryTreeLogicNet.

Computes, for x:[B,256], W_leaf:[256,256], weights:[255,2], biases:[255],
w_out:[1,1], b_out:[1]:

    leaf = sigmoid(x @ W_leaf.T - 2)                       # (B, 256)
    8-level pairwise tree reduce with generalized-gcd nodes # (B, 1)
    out  = sigmoid(root * w_out + b_out)

Key transformations (all host-side constant folding; math exact to ~1e-6):
  - every tree value is positive (sigmoid outputs x positive weights), so
    the |.| is a no-op and min/max are plain min/max;
  - lam*min + (1-lam)*max  =  A*(l+r) + C*max(l,r), A = lam*k, C = k(1-2lam),
    where the consumer weight k of the NEXT level folds in (w_out at root);
  - each level's A further folds into its children's stored scale (sigma
    chain anchored at RHO), so a level is just  node = (l+r) + Chat*max(l,r)
    with Chat = C/A: 4 DVE tensor_tensor ops per level, all fp16 2x mode;
  - levels are stored in bit-reversed node order, which makes every level's
    children the two contiguous halves of the previous buffer, elementwise
    aligned with the outputs: no strided access anywhere;
  - the reference's +EPS contributes k*EPS per node (~1e-6 absolute at the
    root, far below fp16 storage rounding) and is dropped.

Sharding: pure data parallel over the batch dim across 8 cores. x is
transposed (and cast to bf16) on the host so the matmul contraction dim
lands on SBUF partitions and all DMA is contiguous.
"""

import numpy as np

import concourse.bass as bass
import concourse.bacc as bacc
import concourse.mybir as mybir
import concourse.tile as tile
from concourse.bass_utils import run_bass_kernel_spmd

# ---- problem geometry (hardcoded per contract) ----
B, L = 65536, 256
N_CORES = 8
BS = B // N_CORES            # 8192 rows per core
TILES = BS // 128            # 64 tiles of 128 rows
# Tree-group sizes (in 128-row tiles). Small first group so the DVE tree
# starts as soon as possible; big later groups amortize per-op overhead.
GROUP_SIZES = [8, 24, 32]
XSUB = 16                    # x tiles per DMA chunk (pipelining granularity)
RHO = 128.0                  # global pow2 rescale for the A-folded tree

EPS = 1e-6
SHARPNESS = 1.0
BIAS_SHIFT = -2.0

# dtypes (tunable): matmul path and tree path
MM_DT = mybir.dt.bfloat16    # halves x DMA and runs PE at full rate
TREE_DT = mybir.dt.float16   # 16-bit => DVE 2x mode on contiguous tensor ops
CST_DT = mybir.dt.float16    # tree constants (must match tree dtype for tt)

# engine assignment per tree op: 'v' = vector(DVE), 'g' = gpsimd
# ops: s = l+r, mx = max(l,r), q2 = Chat*mx, nd = s+q2
# GPSIMD measured ~2.4 ns/elem on fp16 tensor_tensor (4x slower than DVE's
# 2x mode) and its Pool engine rejects max/min at codegen — all-DVE wins.
ENG_PLAN = {
    li: {"s": "v", "mx": "v", "q1": "v", "q2": "v", "nd": "v"}
    for li in range(8)
}
WINT_ENG = "v"


def _sigmoid(z):
    return 1.0 / (1.0 + np.exp(-z))


def _levels():
    """[(offset, m)] for m = 128, 64, ..., 1 into the weights/biases arrays."""
    out, off, m = [], 0, 128
    while m >= 1:
        out.append((off, m))
        off += m
        m //= 2
    return out


def _bitrev(n):
    """Bit-reversal permutation of 0..n-1 (involution)."""
    bits = n.bit_length() - 1
    out = np.zeros(n, np.int64)
    for j in range(n):
        r, x = 0, j
        for _ in range(bits):
            r = (r << 1) | (x & 1)
            x >>= 1
        out[j] = r
    return out


def prep_consts(weights, biases, w_out):
    """Host-folded per-node constants, A-folded, in bit-reversed order.

    Each node's A = lam*k coefficient is pushed down into its children's
    scales (sigma chain, anchored at sigma_root = RHO for fp16 range), so
    levels 0..6 need only  node = (l + r) + Chat*max(l, r)  with
    Chat = C/A.  The root level keeps explicit A'=A/RHO, C'=C/RHO.

    Level li stores its m output nodes at position q = bitrev(j); with leaves
    stored bit-reversed too, every level's children are the two contiguous
    halves of the previous buffer, elementwise aligned with the outputs.

    Returns (wint[256], Chat_cat[254], a7, c7) in float64 (orig math),
    br-permuted, ready to cast.
    """
    w = weights.astype(np.float64)
    b = biases.astype(np.float64)
    lv = _levels()
    A_lv, C_lv = [], []
    for li, (off, m) in enumerate(lv):
        lam = _sigmoid(b[off : off + m])
        if li + 1 < len(lv):
            noff, nm = lv[li + 1]
            k = np.empty(m, np.float64)
            k[0::2] = w[noff : noff + nm, 0]
            k[1::2] = w[noff : noff + nm, 1]
        else:
            k = np.full(m, float(w_out[0, 0]), np.float64)
        A_lv.append(lam * k)
        C_lv.append(k * (1.0 - 2.0 * lam))
    # sigma chain: sig[li][j] = scale of level-li node j's stored value.
    # Levels 0..6 use the A-folded 4-op form, so each level's A goes into its
    # children's sigma; the root (level 7) keeps its A explicit, so its
    # children carry only the RHO range-rescale.
    sig = [None] * 7
    sig[6] = np.full(2, RHO)
    for li in range(5, -1, -1):
        j = np.arange(128 >> li)
        sig[li] = sig[li + 1][j >> 1] * A_lv[li + 1][j >> 1]
    l_idx = np.arange(256)
    leaf_sig = sig[0][l_idx >> 1] * A_lv[0][l_idx >> 1]

    wint = np.empty(256, np.float64)
    wint[0::2] = w[0:128, 0]
    wint[1::2] = w[0:128, 1]
    wint = (wint * leaf_sig)[_bitrev(256)]

    Chat_parts = [
        (C_lv[li] / A_lv[li])[_bitrev(128 >> li)] for li in range(7)
    ]
    a7 = float(A_lv[7][0] / RHO)
    c7 = float(C_lv[7][0] / RHO)
    return wint, np.concatenate(Chat_parts), a7, c7


def host_emulate(x, W_leaf, weights, biases, w_out, b_out, dtype=np.float32):
    """Pure-numpy emulation of the exact kernel math/layout (for validation)."""
    wint, Chat_cat, a7, c7 = prep_consts(weights, biases, w_out)
    W_perm = W_leaf[_bitrev(256)]  # leaf l lands in column bitrev(l)
    leaf = _sigmoid(
        (x.astype(np.float32) @ W_perm.T.astype(np.float32)) + np.float32(BIAS_SHIFT)
    ).astype(dtype)
    cur = (leaf * wint.astype(dtype)).astype(dtype)
    off = 0
    for li in range(7):
        m = 128 >> li
        l_, r_ = cur[:, 0:m], cur[:, m : 2 * m]
        s = (l_ + r_).astype(dtype)
        mx = np.maximum(l_, r_)
        Ch = Chat_cat[off : off + m].astype(dtype)
        cur = (s + (mx * Ch).astype(dtype)).astype(dtype)
        off += m
    l_, r_ = cur[:, 0:1], cur[:, 1:2]
    s = (l_ + r_).astype(dtype)
    mx = np.maximum(l_, r_)
    cur = ((s * dtype(a7)).astype(dtype) + (mx * dtype(c7)).astype(dtype)).astype(dtype)
    return _sigmoid(cur.astype(np.float32) + np.float32(b_out[0]))


def build_nc(b_out_val, a7, c7):
    """Build the per-core Bass program (SPMD; same NEFF on all cores)."""
    nc = bacc.Bacc("TRN2", target_bir_lowering=False, debug=False)

    xt = nc.dram_tensor("xt", [2, 128, BS], MM_DT, kind="ExternalInput")
    wt = nc.dram_tensor("wt", [128, 2, 256], MM_DT, kind="ExternalInput")
    # cst rows all identical: [wint'(256) | Chat_cat(254) | pad]
    cst = nc.dram_tensor("cst", [128, 512], CST_DT, kind="ExternalInput")
    outp = nc.dram_tensor("out", [128, TILES], mybir.dt.float32, kind="ExternalOutput")

    CHAT_OFF = 256
    XCOLS = XSUB * 128

    with tile.TileContext(nc) as tc:
        with (
            tc.tile_pool(name="const", bufs=1) as constp,
            tc.tile_pool(name="xload", bufs=4) as xp,
            tc.tile_pool(name="leaf", bufs=2) as leafp,
            tc.tile_pool(name="work", bufs=1) as workp,
            tc.tile_pool(name="psum", bufs=8, space="PSUM") as psp,
        ):
            wt_sb = constp.tile([128, 2, 256], MM_DT)
            nc.sync.dma_start(out=wt_sb[:, :, :], in_=wt.ap())
            cst_sb = constp.tile([128, 512], CST_DT)
            nc.sync.dma_start(out=cst_sb[:, :], in_=cst.ap())
            roots = constp.tile([128, TILES], TREE_DT)
            bias_shift = constp.tile([128, 1], mybir.dt.float32)
            nc.vector.memset(bias_shift[:, :], float(BIAS_SHIFT))
            bias_out = constp.tile([128, 1], mybir.dt.float32)
            nc.vector.memset(bias_out[:, :], float(b_out_val))

            def bconst(lo, n, T):
                """cst slice [128, n] broadcast to [128, T, n]."""
                return (
                    cst_sb[:, lo : lo + n]
                    .rearrange("p (o w) -> p o w", o=1)
                    .broadcast_to([128, T, n])
                )

            eng = {"v": nc.vector, "g": nc.gpsimd}

            gstart = 0
            for T in GROUP_SIZES:
                # x-load chunks of <= XSUB tiles (each even, for psum pairs)
                chunks, rem = [], T
                while rem > 0:
                    c = min(XSUB, rem)
                    chunks.append(c)
                    rem -= c
                assert all(c % 2 == 0 for c in chunks)
                leafg = leafp.tile([128, T, 256], TREE_DT, tag="leafg")
                done = 0
                for xsub in chunks:
                    xcols = xsub * 128
                    xoff = gstart + done * 128
                    xa = xp.tile([128, XCOLS], MM_DT, tag="xa")
                    xb = xp.tile([128, XCOLS], MM_DT, tag="xb")
                    nc.sync.dma_start(
                        out=xa[:, 0:xcols], in_=xt.ap()[0, :, xoff : xoff + xcols]
                    )
                    nc.sync.dma_start(
                        out=xb[:, 0:xcols], in_=xt.ap()[1, :, xoff : xoff + xcols]
                    )
                    for tp in range(xsub // 2):
                        ps = psp.tile([128, 2, 256], mybir.dt.float32, tag="ps")
                        for half in range(2):
                            tl = 2 * tp + half  # tile within sub-block
                            bsl = slice(tl * 128, (tl + 1) * 128)
                            nc.tensor.matmul(
                                ps[:, half, :],
                                xa[:, bsl],
                                wt_sb[:, 0, :],
                                start=True,
                                stop=False,
                            )
                            nc.tensor.matmul(
                                ps[:, half, :],
                                xb[:, bsl],
                                wt_sb[:, 1, :],
                                start=False,
                                stop=True,
                            )
                        t0 = done + 2 * tp
                        nc.scalar.activation(
                            out=leafg[:, t0 : t0 + 2, :],
                            in_=ps[:, :, :],
                            func=mybir.ActivationFunctionType.Sigmoid,
                            bias=bias_shift[:, :],
                            scale=float(SHARPNESS),
                        )
                    done += xsub

                # level-0 child weights (sigma-folded): cur = leaf * wint'
                # One storage tile for the whole node chain (level k's output
                # overwrites the front of its input region) and one scratch
                # tile for s/mx/q2 at fixed offsets. All tree ops run on the
                # Vector engine, so program order replaces per-tile semaphore
                # bookkeeping; Tile still tracks the intra-tile WAR hazards.
                cur0 = workp.tile([128, T, 256], TREE_DT, tag="in1", bufs=2)
                scr = workp.tile([128, T, 384], TREE_DT, tag="scratch", bufs=1)
                eng[WINT_ENG].tensor_tensor(
                    out=cur0[:, :, :],
                    in0=leafg[:, :, :],
                    in1=bconst(0, 256, T),
                    op=mybir.AluOpType.mult,
                )

                off = 0
                for li in range(7):
                    m = 128 >> li
                    p = ENG_PLAN[li]
                    le = cur0[:, :, 0:m]
                    ro = cur0[:, :, m : 2 * m]
                    s = scr[:, :, 0:m]
                    mx = scr[:, :, 128 : 128 + m]
                    q2 = scr[:, :, 256 : 256 + m]
                    eng[p["s"]].tensor_tensor(
                        out=s, in0=le, in1=ro, op=mybir.AluOpType.add
                    )
                    eng[p["mx"]].tensor_tensor(
                        out=mx, in0=le, in1=ro, op=mybir.AluOpType.max
                    )
                    eng[p["q2"]].tensor_tensor(
                        out=q2,
                        in0=mx,
                        in1=bconst(CHAT_OFF + off, m, T),
                        op=mybir.AluOpType.mult,
                    )
                    eng[p["nd"]].tensor_tensor(
                        out=cur0[:, :, 0:m],
                        in0=s,
                        in1=q2,
                        op=mybir.AluOpType.add,
                    )
                    off += m

                # root level: explicit A' = A/RHO, C' = C/RHO immediates
                s = scr[:, :, 0:1]
                mx = scr[:, :, 128:129]
                q2 = scr[:, :, 256:257]
                nc.vector.tensor_tensor(
                    out=s,
                    in0=cur0[:, :, 0:1],
                    in1=cur0[:, :, 1:2],
                    op=mybir.AluOpType.add,
                )
                nc.vector.tensor_tensor(
                    out=mx,
                    in0=cur0[:, :, 0:1],
                    in1=cur0[:, :, 1:2],
                    op=mybir.AluOpType.max,
                )
                nc.vector.tensor_scalar_mul(out=q2, in0=mx, scalar1=float(c7))
                rsl = roots[:, gstart // 128 : gstart // 128 + T].rearrange(
                    "p (t o) -> p t o", o=1
                )
                nc.vector.scalar_tensor_tensor(
                    out=rsl,
                    in0=s,
                    scalar=float(a7),
                    in1=q2,
                    op0=mybir.AluOpType.mult,
                    op1=mybir.AluOpType.add,
                )
                gstart += T * 128

            final = constp.tile([128, TILES], mybir.dt.float32)
            nc.scalar.activation(
                out=final[:, :],
                in_=roots[:, :],
                func=mybir.ActivationFunctionType.Sigmoid,
                bias=bias_out[:, :],
                scale=1.0,
            )
            nc.sync.dma_start(out=outp.ap(), in_=final[:, :])

    nc.compile()
    return nc


def make_in_maps(x, W_leaf, weights, biases, w_out):
    """Host-side sharding + layout prep. Returns per-core input dicts."""
    import ml_dtypes

    np_mm = ml_dtypes.bfloat16
    np_cst = np.float16
    wint, Chat_cat, a7, c7 = prep_consts(weights, biases, w_out)

    cst_row = np.zeros(512, np_cst)
    cst_row[0:256] = wint.astype(np_cst)
    cst_row[256 : 256 + 254] = Chat_cat.astype(np_cst)
    cst = np.ascontiguousarray(np.broadcast_to(cst_row, (128, 512)))

    # leaf l lands in column bitrev(l); wt[p, c, l] = W_perm[l, c*128 + p]
    W_perm = W_leaf[_bitrev(256)]
    WT = np.ascontiguousarray(W_perm.T.astype(np_mm))  # [256, 256] (k, l)
    wt_host = np.ascontiguousarray(WT.reshape(2, 128, 256).transpose(1, 0, 2))

    xT = np.ascontiguousarray(x.T.astype(np_mm))  # [256, B]
    in_maps = []
    for c in range(N_CORES):
        sh = np.ascontiguousarray(
            xT[:, c * BS : (c + 1) * BS].reshape(2, 128, BS)
        )
        in_maps.append({"xt": sh, "wt": wt_host, "cst": cst})
    return in_maps, a7, c7


def gather_out(results):
    """Per-core [128, TILES] outputs -> full [B, 1]."""
    full = np.empty((B, 1), np.float32)
    for c in range(N_CORES):
        r = np.asarray(results[c]["out"])  # [128, TILES]
        full[c * BS : (c + 1) * BS, 0] = r.T.reshape(BS)
    return full


def kernel(x, W_leaf, weights, biases, w_out, b_out, _run_kwargs=None):
    x = np.asarray(x, dtype=np.float32)
    W_leaf = np.asarray(W_leaf, dtype=np.float32)
    weights = np.asarray(weights, dtype=np.float32)
    biases = np.asarray(biases, dtype=np.float32)
    w_out = np.asarray(w_out, dtype=np.float32)
    b_out = np.asarray(b_out, dtype=np.float32)
    in_maps, a7, c7 = make_in_maps(x, W_leaf, weights, biases, w_out)
    nc = build_nc(float(b_out[0]), a7, c7)
    kw = dict(_run_kwargs or {})
    res = run_bass_kernel_spmd(nc, in_maps, core_ids=list(range(N_CORES)), **kw)
    out = gather_out(res.results)
    if _run_kwargs is not None:
        kernel.last_results = res
    return out



# revision 5
# speedup vs baseline: 1.1801x; 1.1801x over previous
"""Trainium2 Bass kernel for nn_BinaryTreeLogicNet (v2: custom-DVE level 0).

Math (x:[B,256], W_leaf:[256,256], weights:[255,2], biases:[255],
w_out:[1,1], b_out:[1]):

    leaf = sigmoid(x @ W_leaf.T - 2)                       # (B, 256)
    8-level pairwise tree reduce with generalized-gcd nodes # (B, 1)
    out  = sigmoid(root * w_out + b_out)

All tree values are positive, so each node is
    node = A*(l+r) + C*max(l,r),  A = lam*k, C = k*(1-2*lam)
(k = consumer weight folded in).  Per-core structure (B/8 rows):

  1. Matmul with W stationary and xT streamed ("orientation-2"): psum is
     [leaf-node partitions, batch free].  Leaf pairs are split across two
     psum tiles (left children -> uA, right -> uB) in bit-reversed node
     order, so level-0 is a per-partition op.
  2. ScalarE sigmoid psum->SBUF fp16.
  3. Level 0 runs as ONE custom DVE instruction (TREELEAF:
     out = (C0*in0 + in1) + C1*max(C0*in0, in1), per-partition C0/C1),
     folding the per-leaf weights and the node constants; that is ~3x
     cheaper than the stock wint-mult + 4-op level.
  4. A 4x-mode tensor_scalar rescales v0 to the sigma chain the stock
     levels expect; a DMA xbar transpose moves v0 [128, F] to batch-major
     [128, F/128, 128].
  5. Levels 1-7 run batch-major exactly like the v1 kernel (4 tensor_tensor
     per level on bit-reversed halves; root with explicit A', C').
  6. Final sigmoid(root + b_out) on ScalarE, DMA out.

Sharding: pure data parallel over batch across 8 cores; x transposed and
cast to bf16 on the host so the contraction dim is on partitions.
"""

import numpy as np

import concourse.bass as bass
import concourse.bacc as bacc
import concourse.mybir as mybir
import concourse.tile as tile
from concourse.bass_utils import run_bass_kernel_spmd

# ---- custom DVE op (registered into the concourse catalog at import) ------
import concourse.dve_ops as dve_ops
from concourse.dve_spec import Spec, Src0, Src1, C0, C1, maxx, lower, _has_src1
from concourse.dve_uop import DveOpSpec


def _register_treeleaf():
    name = "TREELEAF_ANT"
    if name in dve_ops._SUB_OPCODE_FOR_NAME:
        for op in dve_ops.OPS:
            if op.name == name:
                return op
        raise RuntimeError(name)
    t = Src0 * C0
    spec = Spec(
        body=(t + Src1) + C1 * maxx(t, Src1),
        reference=lambda in0, in1, s0, s1, imm2: (
            in0.astype(np.float32) * s0 + in1.astype(np.float32)
        )
        + s1 * np.maximum(in0.astype(np.float32) * s0, in1.astype(np.float32)),
    )
    row = dve_ops._CUSTOM_DVE_ROW_BASE + len(dve_ops.OPS)
    assert row < 0x20
    shas = {}
    for ver in ("v3", "v4"):
        s = DveOpSpec(
            name=name, opcode=row, uops=lower(spec, ver=ver), rd1_en=_has_src1(spec)
        )
        shas[ver] = s.sha(ver)
    op = dve_ops.DveOp(name, spec, subdim=False, uops_sha=shas)
    dve_ops.OPS.append(op)
    dve_ops._SUB_OPCODE_FOR_NAME[name] = row
    dve_ops.CUSTOM_DVE_SPECS[name] = spec
    return op


TREELEAF = _register_treeleaf()

# ---- problem geometry (hardcoded per contract) ----
B, L = 65536, 256
N_CORES = 8
BS = B // N_CORES            # 8192 rows per core
TILES = BS // 128            # 64 tiles of 128 rows
SC = 2048                    # super-chunk batch columns
NSC = BS // SC               # 4 super-chunks
PC = 1024                    # psum chunk (2 banks) for matmul/sigmoid
MMF = 512                    # matmul free size per instruction
RHO = 128.0                  # pow2 rescale anchoring the stock sigma chain

EPS = 1e-6
SHARPNESS = 1.0
BIAS_SHIFT = -2.0

MM_DT = mybir.dt.bfloat16
TREE_DT = mybir.dt.float16
CST_DT = mybir.dt.float16


def _sigmoid(z):
    return 1.0 / (1.0 + np.exp(-z))


def _levels():
    out, off, m = [], 0, 128
    while m >= 1:
        out.append((off, m))
        off += m
        m //= 2
    return out


def _bitrev(n):
    bits = n.bit_length() - 1
    out = np.zeros(n, np.int64)
    for j in range(n):
        r, x = 0, j
        for _ in range(bits):
            r = (r << 1) | (x & 1)
            x >>= 1
        out[j] = r
    return out


def prep_consts(weights, biases, w_out):
    """Host-folded constants (float64), all in bit-reversed position order.

    Returns dict with:
      a0[128], ch0[128]  L0 TREELEAF consts
      fix0[128]          v0 rescale onto the stock sigma-chain target
      chat_cat[126]      Chat for levels 1..6, concatenated
      a7, c7             root level explicit consts (on RHO scale)
    """
    w = weights.astype(np.float64)
    b = biases.astype(np.float64)
    lv = _levels()
    A_lv, C_lv, WL, WR = [], [], [], []
    for li, (off, m) in enumerate(lv):
        lam = _sigmoid(b[off : off + m])
        if li + 1 < len(lv):
            noff, nm = lv[li + 1]
            k = np.empty(m, np.float64)
            k[0::2] = w[noff : noff + nm, 0]
            k[1::2] = w[noff : noff + nm, 1]
        else:
            k = np.full(m, float(w_out[0, 0]), np.float64)
        A_lv.append(lam * k)
        C_lv.append(k * (1.0 - 2.0 * lam))
        WL.append(w[off : off + m, 0].copy())
        WR.append(w[off : off + m, 1].copy())

    # stock sigma-chain targets: sig[li][j] = stored scale of level-li node j
    sig = [None] * 7
    sig[6] = np.full(2, RHO)
    for li in range(5, -1, -1):
        j = np.arange(128 >> li)
        sig[li] = sig[li + 1][j >> 1] * A_lv[li + 1][j >> 1]

    # custom L0: v0 = (a0*uA + uB) + ch0*max(.)  => exact0 = sigma0_a * v0
    a0 = WL[0] / WR[0]
    ch0 = C_lv[0] / A_lv[0]
    sigma0_a = A_lv[0] * WR[0]
    # corrected v0' = v0 * fix0 is stored at the stock target scale sig[0]
    fix0 = sigma0_a / sig[0]

    chat_parts = [(C_lv[li] / A_lv[li])[_bitrev(128 >> li)] for li in range(1, 7)]
    a7 = float(A_lv[7][0] / RHO)
    c7 = float(C_lv[7][0] / RHO)
    return {
        "a0": a0[_bitrev(128)],
        "ch0": ch0[_bitrev(128)],
        "fix0": fix0[_bitrev(128)],
        "chat_cat": np.concatenate(chat_parts),
        "a7": a7,
        "c7": c7,
    }


def host_emulate(x, W_leaf, weights, biases, w_out, b_out, dtype=np.float16):
    """Numpy emulation of the kernel math/layout for validation."""
    cst = prep_consts(weights, biases, w_out)
    br128 = _bitrev(128)
    lA = 2 * br128
    lB = lA + 1
    xf = x.astype(np.float32)
    zA = xf @ W_leaf[lA].T.astype(np.float32) + np.float32(BIAS_SHIFT)
    zB = xf @ W_leaf[lB].T.astype(np.float32) + np.float32(BIAS_SHIFT)
    uA = _sigmoid(zA).astype(dtype).astype(np.float32)
    uB = _sigmoid(zB).astype(dtype).astype(np.float32)
    a0 = cst["a0"].astype(np.float32)
    ch0 = cst["ch0"].astype(np.float32)
    t = uA * a0  # fp32 internally in the custom op
    v0 = ((t + uB) + ch0 * np.maximum(t, uB)).astype(dtype)
    v0f = (v0.astype(np.float32) * cst["fix0"].astype(np.float32)).astype(dtype)
    cur = v0f
    off = 0
    for li in range(1, 7):
        m = 128 >> li
        l_, r_ = cur[:, 0:m], cur[:, m : 2 * m]
        s = (l_.astype(np.float32) + r_.astype(np.float32)).astype(dtype)
        mx = np.maximum(l_, r_)
        Ch = cst["chat_cat"][off : off + m].astype(dtype)
        cur = (
            s.astype(np.float32)
            + (mx.astype(np.float32) * Ch.astype(np.float32))
            .astype(dtype)
            .astype(np.float32)
        ).astype(dtype)
        off += m
    l_, r_ = cur[:, 0:1].astype(np.float32), cur[:, 1:2].astype(np.float32)
    s = (l_ + r_).astype(dtype).astype(np.float32)
    mx = np.maximum(l_, r_)
    root = (
        s * np.float32(cst["a7"]) + (mx * np.float32(cst["c7"])).astype(dtype)
    ).astype(np.float32)
    return _sigmoid(root + np.float32(b_out[0]))


def build_nc(b_out_val, a7, c7):
    nc = bacc.Bacc("TRN2", target_bir_lowering=False, debug=False)

    xt = nc.dram_tensor("xt", [2, 128, BS], MM_DT, kind="ExternalInput")
    # 4 stationaries [k 128, {WA0,WA1,WB0,WB1}, j 128]
    wst = nc.dram_tensor("wst", [128, 4, 128], MM_DT, kind="ExternalInput")
    # per-partition consts (fp32): a0, ch0, fix0
    ppc = nc.dram_tensor("ppc", [128, 3], mybir.dt.float32, kind="ExternalInput")
    # batch-major const row: chat_cat(126) | pad, replicated on partitions
    cst = nc.dram_tensor("cst", [128, 128], CST_DT, kind="ExternalInput")
    outp = nc.dram_tensor("out", [128, TILES], mybir.dt.float32, kind="ExternalOutput")

    SCT = SC // 128  # tiles per super-chunk (16)

    with tile.TileContext(nc) as tc:
        with (
            tc.tile_pool(name="const", bufs=1) as constp,
            tc.tile_pool(name="xload", bufs=3) as xp,
            tc.tile_pool(name="u", bufs=2) as up,
            tc.tile_pool(name="v", bufs=2) as vp,
            tc.tile_pool(name="bm", bufs=1) as bmp,
            tc.tile_pool(name="ps", bufs=2, space="PSUM") as psp,
        ):
            wsb = constp.tile([128, 4, 128], MM_DT)
            nc.sync.dma_start(out=wsb[:, :, :], in_=wst.ap())
            ppc_sb = constp.tile([128, 3], mybir.dt.float32)
            nc.sync.dma_start(out=ppc_sb[:, :], in_=ppc.ap())
            cst_sb = constp.tile([128, 128], CST_DT)
            nc.sync.dma_start(out=cst_sb[:, :], in_=cst.ap())
            bias_shift = constp.tile([128, 1], mybir.dt.float32)
            nc.vector.memset(bias_shift[:, :], float(BIAS_SHIFT))
            bias_out = constp.tile([128, 1], mybir.dt.float32)
            nc.vector.memset(bias_out[:, :], float(b_out_val))

            # batch-major storage for the whole core
            v0T = bmp.tile([128, TILES, 128], TREE_DT)
            roots = bmp.tile([128, TILES], TREE_DT)
            scr = bmp.tile([128, TILES, 192], TREE_DT)

            def bconst(lo, n, T, toff):
                return (
                    cst_sb[:, lo : lo + n]
                    .rearrange("p (o w) -> p o w", o=1)
                    .broadcast_to([128, T, n])
                )

            def stock_group(tsl, T):
                """Levels 1..6 + root on v0T[:, tsl, :]."""
                cur = v0T[:, tsl, :]
                off = 0
                for li2 in range(6):
                    m = 64 >> li2
                    le = cur[:, :, 0:m]
                    ro = cur[:, :, m : 2 * m]
                    s = scr[:, tsl, 0:m]
                    mx = scr[:, tsl, 64 : 64 + m]
                    q2 = scr[:, tsl, 128 : 128 + m]
                    nc.vector.tensor_tensor(
                        out=s, in0=le, in1=ro, op=mybir.AluOpType.add
                    )
                    nc.vector.tensor_tensor(
                        out=mx, in0=le, in1=ro, op=mybir.AluOpType.max
                    )
                    nc.vector.tensor_tensor(
                        out=q2,
                        in0=mx,
                        in1=bconst(off, m, T, tsl),
                        op=mybir.AluOpType.mult,
                    )
                    nc.vector.tensor_tensor(
                        out=cur[:, :, 0:m], in0=s, in1=q2, op=mybir.AluOpType.add
                    )
                    off += m
                s = scr[:, tsl, 0:1]
                mx = scr[:, tsl, 64:65]
                q2 = scr[:, tsl, 128:129]
                nc.vector.tensor_tensor(
                    out=s, in0=cur[:, :, 0:1], in1=cur[:, :, 1:2],
                    op=mybir.AluOpType.add,
                )
                nc.vector.tensor_tensor(
                    out=mx, in0=cur[:, :, 0:1], in1=cur[:, :, 1:2],
                    op=mybir.AluOpType.max,
                )
                nc.vector.tensor_scalar_mul(out=q2, in0=mx, scalar1=float(c7))
                rsl = roots[:, tsl].rearrange("p (t o) -> p t o", o=1)
                nc.vector.scalar_tensor_tensor(
                    out=rsl,
                    in0=s,
                    scalar=float(a7),
                    in1=q2,
                    op0=mybir.AluOpType.mult,
                    op1=mybir.AluOpType.add,
                )

            for sc in range(NSC):
                xoff = sc * SC
                xa = xp.tile([128, SC], MM_DT, tag="xa")
                xb = xp.tile([128, SC], MM_DT, tag="xb")
                nc.sync.dma_start(out=xa[:, :], in_=xt.ap()[0, :, xoff : xoff + SC])
                nc.sync.dma_start(out=xb[:, :], in_=xt.ap()[1, :, xoff : xoff + SC])

                uA = up.tile([128, SC], TREE_DT, tag="uA")
                uB = up.tile([128, SC], TREE_DT, tag="uB")
                for pc in range(SC // PC):
                    po = pc * PC
                    psA = psp.tile([128, PC], mybir.dt.float32, tag="psA")
                    psB = psp.tile([128, PC], mybir.dt.float32, tag="psB")
                    for half, ps in ((0, psA), (1, psB)):
                        for ki in range(2):
                            xsrc = xa if ki == 0 else xb
                            st = wsb[:, 2 * half + ki, :]
                            for f in range(PC // MMF):
                                fo = po + f * MMF
                                nc.tensor.matmul(
                                    ps[:, f * MMF : (f + 1) * MMF],
                                    st,
                                    xsrc[:, fo : fo + MMF],
                                    start=(ki == 0),
                                    stop=(ki == 1),
                                )
                    nc.scalar.activation(
                        out=uA[:, po : po + PC],
                        in_=psA[:, :],
                        func=mybir.ActivationFunctionType.Sigmoid,
                        bias=bias_shift[:, :],
                        scale=float(SHARPNESS),
                    )
                    nc.scalar.activation(
                        out=uB[:, po : po + PC],
                        in_=psB[:, :],
                        func=mybir.ActivationFunctionType.Sigmoid,
                        bias=bias_shift[:, :],
                        scale=float(SHARPNESS),
                    )

                # L0 custom: v0 = (a0*uA + uB) + ch0*max(a0*uA, uB)
                v0 = vp.tile([128, SC], TREE_DT, tag="v0")
                nc.vector._custom_dve(
                    TREELEAF,
                    out=v0[:, :],
                    in0=uA[:, :],
                    in1=uB[:, :],
                    s0=ppc_sb[:, 0:1],
                    s1=ppc_sb[:, 1:2],
                )
                # rescale onto the stock sigma chain (tensor_scalar, 4x)
                v0f = vp.tile([128, SC], TREE_DT, tag="v0f")
                nc.vector.tensor_scalar(
                    out=v0f[:, :],
                    in0=v0[:, :],
                    scalar1=ppc_sb[:, 2:3],
                    scalar2=None,
                    op0=mybir.AluOpType.mult,
                )
                # transpose [128, SC] -> batch-major [128, SCT, 128]
                nc.sync.dma_start_transpose(
                    v0T[:, sc * SCT : (sc + 1) * SCT, :], v0f[:, :]
                )
                # stock levels for this super-chunk's tiles
                stock_group(slice(sc * SCT, (sc + 1) * SCT), SCT)

            final = constp.tile([128, TILES], mybir.dt.float32)
            nc.scalar.activation(
                out=final[:, :],
                in_=roots[:, :],
                func=mybir.ActivationFunctionType.Sigmoid,
                bias=bias_out[:, :],
                scale=1.0,
            )
            nc.sync.dma_start(out=outp.ap(), in_=final[:, :])

    nc.compile()
    return nc


def make_in_maps(x, W_leaf, weights, biases, w_out):
    import ml_dtypes

    np_mm = ml_dtypes.bfloat16
    cst = prep_consts(weights, biases, w_out)
    br128 = _bitrev(128)
    lA = 2 * br128
    lB = lA + 1

    Wf = W_leaf.astype(np.float32)
    WA = Wf[lA]  # [128 j, 256 k]
    WB = Wf[lB]
    wst = np.empty((128, 4, 128), np.float32)
    wst[:, 0, :] = WA[:, 0:128].T
    wst[:, 1, :] = WA[:, 128:256].T
    wst[:, 2, :] = WB[:, 0:128].T
    wst[:, 3, :] = WB[:, 128:256].T
    wst = np.ascontiguousarray(wst.astype(np_mm))

    ppc = np.zeros((128, 3), np.float32)
    ppc[:, 0] = cst["a0"]
    ppc[:, 1] = cst["ch0"]
    ppc[:, 2] = cst["fix0"]

    cst_row = np.zeros(128, np.float16)
    cst_row[0:126] = cst["chat_cat"].astype(np.float16)
    cst_np = np.ascontiguousarray(np.broadcast_to(cst_row, (128, 128)))

    xT = np.ascontiguousarray(x.T.astype(np_mm))  # [256, B]
    in_maps = []
    for c in range(N_CORES):
        sh = np.ascontiguousarray(xT[:, c * BS : (c + 1) * BS].reshape(2, 128, BS))
        in_maps.append({"xt": sh, "wst": wst, "ppc": ppc, "cst": cst_np})
    return in_maps, cst["a7"], cst["c7"]


def gather_out(results):
    full = np.empty((B, 1), np.float32)
    for c in range(N_CORES):
        r = np.asarray(results[c]["out"])  # [128, TILES]
        full[c * BS : (c + 1) * BS, 0] = r.T.reshape(BS)
    return full


def kernel(x, W_leaf, weights, biases, w_out, b_out, _run_kwargs=None):
    x = np.asarray(x, dtype=np.float32)
    W_leaf = np.asarray(W_leaf, dtype=np.float32)
    weights = np.asarray(weights, dtype=np.float32)
    biases = np.asarray(biases, dtype=np.float32)
    w_out = np.asarray(w_out, dtype=np.float32)
    b_out = np.asarray(b_out, dtype=np.float32)
    in_maps, a7, c7 = make_in_maps(x, W_leaf, weights, biases, w_out)
    nc = build_nc(float(b_out[0]), a7, c7)
    kw = dict(_run_kwargs or {})
    res = run_bass_kernel_spmd(nc, in_maps, core_ids=list(range(N_CORES)), **kw)
    out = gather_out(res.results)
    if _run_kwargs is not None:
        kernel.last_results = res
    return out
